# revision 26
# baseline (speedup 1.0000x reference)
# BinaryTreeLSTM forest kernel for 8 trn2 NeuronCores (Bass/Tile).
#
# Strategy: data-parallel over trees.  Each core owns 6 whole trees (trees
# 0..47) plus one depth-8 subtree of tree 48 (cores 0-3) or 49 (cores 4-7),
# giving all 8 cores identical, perfectly balanced work (12793 nodes).  The 6
# remaining nodes (levels 9-10 of trees 48/49) are combined on the host
# during unshard.  Within a core, nodes are laid out level-major across
# participants; every participant's level size halves going down, so the
# children of parent column p at level l are exactly columns 2p, 2p+1 of
# level l-1 -- all per-level GEMMs run on contiguous column ranges.
# Activations live transposed ([feature, node]): stationary operand = weight
# tile, moving operand = activation columns, PSUM accumulates x-part +
# children-part + bias (rank-1 matmul).  GEMMs in fp16 (same PE throughput
# as bf16, 8x finer mantissa); cell state c in fp32.
import numpy as np
import ml_dtypes  # noqa: F401  (env dependency of concourse)

TREES, DEPTH, H = 50, 10, 512
PER = 2 ** (DEPTH + 1) - 1          # 2047 nodes per tree
N = TREES * PER                     # 102350
NCORES = 8
NLEV = DEPTH + 1
NL = [2 ** (DEPTH - l) for l in range(NLEV)]          # per-tree level sizes
SL = [0]
for _n in NL:
    SL.append(SL[-1] + _n)                            # per-tree level starts

SUB_D = 8                                             # subtree root level
PL = [6 * NL[l] + (2 ** (SUB_D - l) if l <= SUB_D else 0) for l in range(NLEV)]
LL = [0]
for _p in PL[:-1]:
    LL.append(LL[-1] + _p)                            # per-core level col starts
NCOLS = sum(PL)                     # 12793
NPAD = 12800
CH = 512                            # parents per chunk
RES_FROM = 5                        # levels >= RES_FROM read children from SBUF
HB_COLS = LL[RES_FROM - 1] + PL[RES_FROM - 1]         # staged h-fp16 cols

_BUILT = {}


def _build_kernel(repeat=1):
    """Build + compile the per-core SPMD Bass program."""
    import contextlib

    import concourse.bass as bass  # noqa: F401
    import concourse.mybir as mybir
    import concourse.tile as tile
    from concourse import bacc

    dt = mybir.dt
    Sig = mybir.ActivationFunctionType.Sigmoid
    Tanh = mybir.ActivationFunctionType.Tanh

    nc = bacc.Bacc("TRN2", target_bir_lowering=False, debug=False)

    xT = nc.dram_tensor("xT", [512, NPAD], dt.float16, kind="ExternalInput").ap()
    wiou = nc.dram_tensor("wiou", [512, 1536], dt.float16, kind="ExternalInput").ap()
    ucat = nc.dram_tensor("ucat", [1024, 1536], dt.float16, kind="ExternalInput").ap()
    wf = nc.dram_tensor("wf", [512, 512], dt.float16, kind="ExternalInput").ap()
    ufc = nc.dram_tensor("ufc", [512, 1024], dt.float16, kind="ExternalInput").ap()
    # biases laid out [partition, gate-tile] so they feed the ScalarE
    # activation bias operand (per-partition scalar) directly
    biou = nc.dram_tensor("biou", [128, 12], dt.float32, kind="ExternalInput").ap()
    bfw = nc.dram_tensor("bfw", [128, 4], dt.float32, kind="ExternalInput").ap()
    hT = nc.dram_tensor("hT", [512, NPAD], dt.float32, kind="ExternalOutput").ap()
    cT = nc.dram_tensor("cT", [512, NPAD], dt.float32, kind="ExternalOutput").ap()
    hb16 = nc.dram_tensor("hb16", [512, HB_COLS], dt.float16).ap()

    xTr = xT.rearrange("(kt kp) n -> kp kt n", kp=128)
    hTr = hT.rearrange("(kt kp) n -> kp kt n", kp=128)
    cTr = cT.rearrange("(kt kp) n -> kp kt n", kp=128)
    hbr = hb16.rearrange("(kt kp) n -> kp kt n", kp=128)

    with tile.TileContext(nc) as tc:
        with (
            tc.tile_pool(name="w", bufs=1) as wp,
            tc.tile_pool(name="persist", bufs=1) as pp,
            tc.tile_pool(name="xin", bufs=3) as xp,
            tc.tile_pool(name="chld", bufs=2) as chp,
            tc.tile_pool(name="gate", bufs=5) as gp,
            tc.tile_pool(name="fg", bufs=5) as fp,
            tc.tile_pool(name="fsb", bufs=3) as fsp,
            tc.tile_pool(name="outp", bufs=2) as op,
            tc.tile_pool(name="ps", bufs=4, space="PSUM") as psp,
        ):
            # W_iou + bias first (level 0 only needs these); recurrence
            # weights stream in per k-tile behind the first x chunks.
            wiou_sb = wp.tile([128, 4, 1536], dt.float16)
            nc.sync.dma_start(wiou_sb[:], wiou.rearrange("(kt kp) m -> kp kt m", kp=128))
            biou_sb = wp.tile([128, 12], dt.float32)
            nc.sync.dma_start(biou_sb[:], biou[:])
            bf_sb = wp.tile([128, 4], dt.float32)
            nc.sync.dma_start(bf_sb[:], bfw[:])
            # Recurrence weights aren't needed until level 1 -- load them on
            # the SWDGE (gpsimd) queues so level 0's x-chunk DMAs don't queue
            # behind 5 MB of weights on the HWDGE path.
            ucat_sb = wp.tile([128, 8, 1536], dt.float16)
            ucat_r = ucat.rearrange("(kt kp) m -> kp kt m", kp=128)
            for kt in range(8):
                nc.gpsimd.dma_start(ucat_sb[:, kt], ucat_r[:, kt])
            wf_sb = wp.tile([128, 4, 512], dt.float16)
            nc.gpsimd.dma_start(wf_sb[:], wf.rearrange("(kt kp) m -> kp kt m", kp=128))
            ufc_sb = wp.tile([128, 4, 1024], dt.float16)
            ufc_r = ufc.rearrange("(kt kp) m -> kp kt m", kp=128)
            for kt in range(4):
                nc.gpsimd.dma_start(ufc_sb[:, kt], ufc_r[:, kt])

            # SBUF-resident children h/c for small levels; level l writes slot
            # l % 2, level l+1 reads it.
            sz = [0, 0]
            sz[(RES_FROM - 1) % 2] = PL[RES_FROM - 1]
            sz[RES_FROM % 2] = PL[RES_FROM]
            slot_h = [
                pp.tile([128, 4, sz[0]], dt.float16, name="sh0"),
                pp.tile([128, 4, sz[1]], dt.float16, name="sh1"),
            ]
            slot_c = [
                pp.tile([128, 4, sz[0]], dt.float32, name="sc0"),
                pp.tile([128, 4, sz[1]], dt.float32, name="sc1"),
            ]

            _rep = contextlib.ExitStack()
            if repeat > 1:
                _rep.enter_context(tc.For_i(0, repeat, 1))
            for l in range(NLEV):
                P = PL[l]
                for c0 in range(0, P, CH):
                    ch = min(CH, P - c0)
                    ch2 = 2 * ch
                    cols = slice(LL[l] + c0, LL[l] + c0 + ch)

                    x_sb = xp.tile([128, 4, CH], dt.float16, tag="x")
                    nc.sync.dma_start(x_sb[:, :, :ch], xTr[:, :, cols])

                    if l > 0:
                        if l < RES_FROM:
                            hch = chp.tile([128, 4, 2 * CH], dt.float16, tag="hch")
                            cch = chp.tile([128, 4, 2 * CH], dt.float32, tag="cch")
                            ccols = slice(LL[l - 1] + 2 * c0, LL[l - 1] + 2 * c0 + ch2)
                            nc.sync.dma_start(hch[:, :, :ch2], hbr[:, :, ccols])
                            nc.sync.dma_start(cch[:, :, :ch2], cTr[:, :, ccols])
                            coff = 0
                        else:
                            hch = slot_h[(l - 1) % 2]
                            cch = slot_c[(l - 1) % 2]
                            coff = 2 * c0

                    # ---- iou = W_iou @ x + U_l @ hL + U_r @ hR + b_iou ----
                    # One PSUM gate-pair tile per 2 gate-tiles; i/o/u = gate
                    # tiles 0-3 / 4-7 / 8-11.
                    gates = []
                    for cls, func in ((0, Sig), (1, Sig), (2, Tanh)):
                        gt = gp.tile([128, 4, CH], dt.float16, tag="g", name="gt")
                        for pq in range(2):
                            ps = psp.tile([128, 2, CH], dt.float32, tag="ps", name="psio")
                            for j in range(2):
                                g = cls * 4 + pq * 2 + j
                                out = ps[:, j, :ch]
                                gsl = slice(g * 128, (g + 1) * 128)
                                for kt in range(4):
                                    nc.tensor.matmul(
                                        out, wiou_sb[:, kt, gsl], x_sb[:, kt, :ch],
                                        start=(kt == 0),
                                        stop=(kt == 3 and l == 0),
                                    )
                                if l > 0:
                                    for kt in range(8):
                                        if kt < 4:
                                            rhs = hch[:, kt, coff : coff + ch2 : 2]
                                        else:
                                            rhs = hch[:, kt - 4, coff + 1 : coff + ch2 : 2]
                                        nc.tensor.matmul(
                                            out, ucat_sb[:, kt, gsl], rhs,
                                            start=False,
                                            stop=(kt == 7),
                                        )
                            for j in range(2):
                                g = cls * 4 + pq * 2 + j
                                nc.scalar.activation(
                                    gt[:, 2 * pq + j, :ch], ps[:, j, :ch], func,
                                    bias=biou_sb[:, g : g + 1],
                                )
                        gates.append(gt)
                    si, so, tu = gates

                    persist = RES_FROM - 1 <= l <= 9
                    if persist:
                        cn = slot_c[l % 2][:, :, c0 : c0 + ch]
                    else:
                        cn_t = op.tile([128, 4, CH], dt.float32, tag="cn", name="cn")
                        cn = cn_t[:, :, :ch]

                    if l == 0:
                        nc.vector.tensor_mul(out=cn, in0=si[:, :, :ch], in1=tu[:, :, :ch])
                    else:
                        t1 = op.tile([128, 4, CH], dt.float16, tag="t1")
                        nc.vector.tensor_mul(
                            out=t1[:, :, :ch], in0=si[:, :, :ch], in1=tu[:, :, :ch]
                        )
                        # ---- forget gates, in halves of <=512 children ----
                        for hh in range(2):
                            chh = min(2 * CH, ch2) - hh * CH
                            if chh <= 0:
                                break
                            chh = min(chh, CH)
                            cb = coff + hh * CH          # child col base
                            pb = hh * (CH // 2)          # parent col base
                            php = chh // 2               # parents in this half
                            # shared S = W_f @ x_rep + b_f, one tile per gate pair
                            s_sb = []
                            for pq in range(2):
                                ps_s = psp.tile(
                                    [128, 2, CH], dt.float32, tag="ps", name="ps_s"
                                )
                                for j in range(2):
                                    g = 2 * pq + j
                                    out = ps_s[:, j, :chh]
                                    gsl = slice(g * 128, (g + 1) * 128)
                                    for kt in range(4):
                                        xrep = x_sb[
                                            :, kt, pb : pb + php, None
                                        ].to_broadcast([128, php, 2])
                                        nc.tensor.matmul(
                                            out, wf_sb[:, kt, gsl], xrep,
                                            start=(kt == 0), stop=(kt == 3),
                                        )
                                ss = fsp.tile(
                                    [128, 2, CH], dt.float16, tag="fs", name="ss"
                                )
                                nc.scalar.copy(ss[:, :, :chh], ps_s[:, :, :chh])
                                s_sb.append(ss)
                            # per-variant U GEMMs; f tile t: v=t//2, gates 2(t%2)..+1
                            f_sb = []
                            for t in range(4):
                                v, pq = t // 2, t % 2
                                psf = psp.tile([128, 2, CH], dt.float32, tag="ps")
                                for j in range(2):
                                    g = 2 * pq + j
                                    out = psf[:, j, :chh]
                                    usl = slice((v * 4 + g) * 128, (v * 4 + g + 1) * 128)
                                    for kt in range(4):
                                        nc.tensor.matmul(
                                            out, ufc_sb[:, kt, usl],
                                            hch[:, kt, cb : cb + chh],
                                            start=(kt == 0), stop=(kt == 3),
                                        )
                                fs = fp.tile([128, 2, CH], dt.float16, tag="f")
                                nc.vector.tensor_add(
                                    out=fs[:, :, :chh],
                                    in0=psf[:, :, :chh],
                                    in1=s_sb[pq][:, :, :chh],
                                )
                                for j in range(2):
                                    g = 2 * pq + j
                                    nc.scalar.activation(
                                        fs[:, j, :chh], fs[:, j, :chh], Sig,
                                        bias=bf_sb[:, g : g + 1],
                                    )
                                f_sb.append(fs)
                            # m = (f_l + f_r) * c_child; cn = t1 + m_even + m_odd
                            for pq in range(2):
                                a, b = f_sb[pq], f_sb[pq + 2]
                                nc.vector.tensor_add(
                                    out=a[:, :, :chh], in0=a[:, :, :chh],
                                    in1=b[:, :, :chh],
                                )
                                nc.vector.tensor_mul(
                                    out=a[:, :, :chh], in0=a[:, :, :chh],
                                    in1=cch[:, 2 * pq : 2 * pq + 2, cb : cb + chh],
                                )
                                gsl2 = slice(2 * pq, 2 * pq + 2)
                                nc.vector.tensor_add(
                                    out=cn[:, gsl2, pb : pb + php],
                                    in0=t1[:, gsl2, pb : pb + php],
                                    in1=a[:, :, 0 : chh : 2],
                                )
                                nc.vector.tensor_add(
                                    out=cn[:, gsl2, pb : pb + php],
                                    in0=cn[:, gsl2, pb : pb + php],
                                    in1=a[:, :, 1 : chh : 2],
                                )

                    tcn = gp.tile([128, 4, CH], dt.float16, tag="g", name="tcn")
                    nc.scalar.activation(tcn[:, :, :ch], cn, Tanh)
                    # h in fp16: the mul writes straight into the persistence
                    # slot (no cast-copy); hT f32 output via casting SWDGE DMA
                    if persist:
                        hn = slot_h[l % 2][:, :, c0 : c0 + ch]
                    else:
                        hn_t = op.tile([128, 4, CH], dt.float16, tag="hn", name="hn")
                        hn = hn_t[:, :, :ch]
                    nc.vector.tensor_mul(out=hn, in0=so[:, :, :ch], in1=tcn[:, :, :ch])
                    if l <= RES_FROM - 2:
                        nc.sync.dma_start(hbr[:, :, cols], hn)
                    nc.gpsimd.dma_start(hTr[:, :, cols], hn)
                    nc.sync.dma_start(cTr[:, :, cols], cn)
            _rep.close()

    nc.compile()
    return nc


def _perm():
    """perm[c, j] = global node id for core c's column j (N = zero pad)."""
    perm = np.full((NCORES, NPAD), N, np.int64)
    for c in range(NCORES):
        pieces = []
        sub_tree = 48 + c // 4          # tree providing this core's subtree
        sub_idx = c % 4                 # which depth-8 subtree of it
        for l in range(NLEV):
            for t in range(6):
                g = c * 6 + t
                pieces.append(g * PER + SL[l] + np.arange(NL[l], dtype=np.int64))
            if l <= SUB_D:
                w = 2 ** (SUB_D - l)    # subtree's node count at this level
                base = sub_tree * PER + SL[l] + sub_idx * w
                pieces.append(base + np.arange(w, dtype=np.int64))
        perm[c, :NCOLS] = np.concatenate(pieces)
    return perm


def _host_tops(h, cc, feats, W_iou, b_iou, U_il, U_ir, W_f, b_f, U_fl, U_fr):
    """Levels 9-10 of trees 48/49 (6 nodes): combine from subtree roots."""
    sig = lambda x: 1.0 / (1.0 + np.exp(-x))  # noqa: E731
    for g in (48, 49):
        for l in (9, 10):
            nodes = g * PER + SL[l] + np.arange(NL[l])
            child = g * PER + SL[l - 1] + np.arange(2 * NL[l])
            x = feats[nodes]
            ch = h[child]
            cc_ch = cc[child]
            iou = (
                x @ W_iou.T + b_iou
                + ch[0::2] @ U_il.T
                + ch[1::2] @ U_ir.T
            )
            i_, o_, u_ = np.split(iou, 3, axis=1)
            wfx = np.repeat(x @ W_f.T + b_f, 2, axis=0)
            fl = sig(wfx + ch @ U_fl.T)
            fr = sig(wfx + ch @ U_fr.T)
            fc = (fl + fr) * cc_ch
            cn = sig(i_) * np.tanh(u_) + fc.reshape(-1, 2, H).sum(1)
            hn = sig(o_) * np.tanh(cn)
            h[nodes] = hn
            cc[nodes] = cn


def kernel(**inputs):
    from concourse.bass_utils import run_bass_kernel_spmd

    if "nc" not in _BUILT:
        _BUILT["nc"] = _build_kernel()
        _BUILT["perm"] = _perm()
    nc = _BUILT["nc"]
    perm = _BUILT["perm"]

    fp16 = np.float16
    f32 = np.float32
    feats = np.asarray(inputs["features"], dtype=f32)
    W_iou = np.asarray(inputs["W_iou_w"], dtype=f32)
    b_iou = np.asarray(inputs["W_iou_b"], dtype=f32)
    U_il = np.asarray(inputs["U_iou_l"], dtype=f32)
    U_ir = np.asarray(inputs["U_iou_r"], dtype=f32)
    W_f = np.asarray(inputs["W_f_w"], dtype=f32)
    b_f = np.asarray(inputs["W_f_b"], dtype=f32)
    U_fl = np.asarray(inputs["U_f_l"], dtype=f32)
    U_fr = np.asarray(inputs["U_f_r"], dtype=f32)

    wshared = {
        "wiou": np.ascontiguousarray(W_iou.T).astype(fp16),
        "ucat": np.ascontiguousarray(
            np.concatenate([U_il.T, U_ir.T], axis=0)
        ).astype(fp16),
        "wf": np.ascontiguousarray(W_f.T).astype(fp16),
        "ufc": np.ascontiguousarray(
            np.concatenate([U_fl.T, U_fr.T], axis=1)
        ).astype(fp16),
        "biou": np.ascontiguousarray(b_iou.reshape(12, 128).T),
        "bfw": np.ascontiguousarray(b_f.reshape(4, 128).T),
    }

    fpad = np.concatenate([feats, np.zeros((1, H), f32)], axis=0)
    in_maps = []
    for c in range(NCORES):
        xc = fpad[perm[c]]                       # [NPAD, 512]
        in_maps.append(
            {"xT": np.ascontiguousarray(xc.T).astype(fp16), **wshared}
        )

    _BUILT["in_maps"] = in_maps
    res = run_bass_kernel_spmd(nc, in_maps, list(range(NCORES)))

    h = np.zeros((N, H), f32)
    cc = np.zeros((N, H), f32)
    for c in range(NCORES):
        mask = perm[c] < N
        idx = perm[c][mask]
        h[idx] = res.results[c]["hT"].T[mask]
        cc[idx] = res.results[c]["cT"].T[mask]
    _host_tops(h, cc, feats, W_iou, b_iou, U_il, U_ir, W_f, b_f, U_fl, U_fr)
    return h, cc
